# revision 18
# baseline (speedup 1.0000x reference)
"""Trainium2 Bass kernel for nn_SUEPNet (gnn_message_passing).

Model: pf_encode (4->16->16 MLP with ELU) -> 2x DynamicEdgeConv (shared
weights, per-event kNN k=8 over 512 particles/event) -> per-event mean pool
-> 16->8->4->1 head.  B=256 events, data-parallel over 8 NeuronCores
(32 events/core, processed as 4 blocks of 8 events).

Key algebraic tricks (all exact up to fp reassociation):
  * EdgeConv edge-MLP max-aggregation commutes with the (monotone) ELU:
      max_k elu([x_i, x_j-x_i] @ Wc + bc)
        = elu(A_i + max_{j in knn(i)} B_j + bc')
    with A = h @ (Wc_top - Wc_bot), B = h @ Wc_bot.  No per-edge work.
  * elu(z) + 1 = relu(z) + min(exp(z), 1); the +1 shift is absorbed into
    the next layer's bias (translation does not change pairwise distances),
    so every ELU costs 2 ACT ops + 1 fused DVE op.
  * kNN ranking: top-8 by (G[i,j] - |h_j|^2/2) == top-8 by -d^2.  The
    -|h_j|^2/2 row is accumulated into PSUM with a K=1 matmul.
  * top-8 per row via the DVE Max8 / FindIndex8 instructions; neighbor
    feature gather via the GPSIMD ap_gather ucode op (per-16-partition
    index lists == per-event index lists).

Self-contained: hardcodes all shapes; only imports concourse (bass).
"""

import functools
import sys

import numpy as np

try:  # noqa: SIM105
    import concourse  # noqa: F401
except ImportError:
    sys.path.insert(0, "/opt/trn_rl_repo")

import concourse.bass as bass
import concourse.mybir as mybir
import concourse.tile as tile
from concourse import library_config

B, NPER, KNN, H = 256, 512, 8, 16
NCORES = 8
EPC = B // NCORES          # events per core = 32
EVB = 8                    # events per block (8 x 16 = 128 partitions)
NBLK = EPC // EVB          # blocks per core = 4

f32 = mybir.dt.float32
f32r = mybir.dt.float32r
u16 = mybir.dt.uint16
i16 = mybir.dt.int16
AF = mybir.ActivationFunctionType
OP = mybir.AluOpType


def _r(ap):
    """View an fp32 AP as float32r for full-rate PE matmuls."""
    return ap.bitcast(f32r)


def build_nc():
    nc = bass.Bass("TRN2", target_bir_lowering=False, debug=False)

    dr = {}

    def din(name, shape, dt=f32):
        dr[name] = nc.dram_tensor(name, shape, dt, kind="ExternalInput").ap()
        return dr[name]

    xt = din("xt", [NBLK, 32, NPER])          # per-block transposed inputs
    din("w1bd", [32, 128])                    # blockdiag W1 (4->16) x8
    din("w2bd", [128, 128])                   # blockdiag W2 (16->16) x8
    din("wabd", [128, 128])                   # blockdiag (Wc_top - Wc_bot) x8
    din("wbbd", [128, 128])                   # blockdiag Wc_bot x8
    din("ones16bd", [128, 8])                 # blockdiag ones(16,1) x8
    din("onesrow", [1, 128])                  # all-ones row
    din("ones4096", [1, EVB * NPER])          # aug ones row for hteS
    din("b1col", [128, 1])
    din("b2pcol", [128, 1])
    din("bcpcol", [128, 1])
    din("wo1s", [16, 8])
    din("wo2", [8, 4])
    din("wo3", [4, 1])
    din("bo1p", [8, 1])
    din("bo2p", [4, 1])
    din("bo3p", [1, 1])
    out_dram = nc.dram_tensor("out", [EPC], f32, kind="ExternalOutput").ap()

    with tile.TileContext(nc) as tc:
        _body(nc, tc, dr, xt, out_dram)
    # raw Bass does not lower extended-inst InstISA subclasses (ap_gather,
    # library reload) to .instr bytes; without this walrus codegen fails
    # with "ISA wrong length".
    from concourse import library_overlay
    library_overlay.lower_extended_insts(nc)
    _split_matmul_waits(nc)
    return nc


# Hardware sync-wait slots per lowered instruction struct: walrus attaches
# all of a Matmult's waits to its LDWEIGHTS (1 slot); HWDGE DMA descriptors
# have 2.  Excess waits are moved to NoOps on the same engine sequencer.
_WAIT_LIMITS = {"Matmult": 1, "DMACopy": 1, "Max": 1, "MaxIndex": 1,
                "TensorScalarPtr": 1, "TensorReduce": 1, "Activation": 1,
                "TensorTensor": 1, "Memset": 1, "TensorCopy": 1, "ISA": 1, "Drain": 1}


def _split_matmul_waits(nc):
    """Move sem waits beyond an instruction's HW wait-slot budget onto
    same-engine NoOps inserted just before it (one wait per NoOp)."""
    for f in nc.m.functions:
        for bb in f.blocks:
            new = []
            for inst in bb.instructions:
                si = inst.sync_info
                limit = _WAIT_LIMITS.get(inst.opcode)
                if (limit is not None and si is not None and si.on_wait
                        and len(si.on_wait) > limit):
                    extra = list(si.on_wait[:-limit])
                    for w in extra:
                        nop = mybir.InstNoOp(
                            name=nc.get_next_instruction_name(), ins=[], outs=[])
                        nop.engine = inst.engine
                        nop.sync_info = mybir.SyncInfo(on_wait=[w], on_update=[])
                        nc.register_instruction(nop)
                        new.append(nop)
                    inst.sync_info = mybir.SyncInfo(
                        on_wait=list(si.on_wait[-limit:]),
                        on_update=list(si.on_update))
                new.append(inst)
            bb.instructions[:] = new


def _body(nc, tc, dr, xt, out_dram):
    import contextlib
    with contextlib.ExitStack() as ctx:
        _body2(nc, tc, dr, xt, out_dram, ctx)


def _body2(nc, tc, dr, xt, out_dram, ctx):
    cp = ctx.enter_context(tc.tile_pool(name="consts", bufs=1))
    mp = ctx.enter_context(tc.tile_pool(name="main", bufs=2))
    sp = ctx.enter_context(tc.tile_pool(name="small", bufs=4))
    pa = ctx.enter_context(tc.tile_pool(name="psA", bufs=3, space="PSUM"))
    pg = ctx.enter_context(tc.tile_pool(name="psG", bufs=5, space="PSUM"))
    dp = ctx.enter_context(tc.tile_pool(name="dram", bufs=4, space="DRAM"))

    # gpsimd ucode library for ap_gather (must precede any ap_gather use)
    nc.gpsimd.load_library(library_config.ap_gather)

    # ---- constants -------------------------------------------------------
    c = {}
    for nm, shape in [
        ("w1bd", [32, 128]), ("w2bd", [128, 128]), ("wabd", [128, 128]),
        ("wbbd", [128, 128]), ("ones16bd", [128, 8]), ("onesrow", [1, 128]),
        ("b1col", [128, 1]), ("b2pcol", [128, 1]), ("bcpcol", [128, 1]),
        ("wo1s", [16, 8]), ("wo2", [8, 4]), ("wo3", [4, 1]),
        ("ones4096", [1, EVB * NPER]),
        ("bo1p", [8, 1]), ("bo2p", [4, 1]), ("bo3p", [1, 1]),
    ]:
        t = cp.tile(shape, f32, tag=f"c_{nm}")
        if nm in ("onesrow",):
            nc.scalar.dma_start(out=_r(t[:]), in_=_r(dr[nm]))
        else:
            nc.scalar.dma_start(out=t[:], in_=dr[nm])
        c[nm] = t

    pooledT = cp.tile([16, 32], f32, tag="pooledT")  # [h, event] per core

    def shifted_elu(z_psum, bias_col, tag, round_out=False):
        """return sbuf tile = elu(z + bias) + 1 = relu(z+b) + min(exp(z+b),1)."""
        p = z_psum.shape[0]
        n = z_psum.shape[-1]
        u = mp.tile([p, n], f32, tag=f"u_{tag}")
        r = mp.tile([p, n], f32, tag=f"r_{tag}")
        nc.scalar.activation(u[:], z_psum, AF.Exp, bias=bias_col, scale=1.0)
        nc.scalar.activation(r[:], z_psum, AF.Relu, bias=bias_col, scale=1.0)
        o = mp.tile([p, n], f32, tag=f"selu_{tag}")
        oap = _r(o[:]) if round_out else o[:]
        nc.vector.scalar_tensor_tensor(
            out=oap, in0=u[:], scalar=1.0, in1=r[:], op0=OP.min, op1=OP.add)
        return o

    def encoder(blk):
        xtile = mp.tile([32, NPER], f32, tag="xt")
        nc.scalar.dma_start(out=xtile[:], in_=xt[blk])
        z1 = pa.tile([128, NPER], f32, tag="ps_aux")
        nc.tensor.matmul(z1[:], c["w1bd"][:], xtile[:])
        h1 = shifted_elu(z1[:], c["b1col"][:], "h1")
        z2 = pa.tile([128, NPER], f32, tag="ps_aux")
        nc.tensor.matmul(z2[:], c["w2bd"][:], h1[:])
        return shifted_elu(z2[:], c["b2pcol"][:], "h2")

    def conv(hT, blk, cv):
        """One DynamicEdgeConv on an 8-event block. hT: [128,512] sbuf."""
        # |h_j|^2 per event -> negsq [8,512] = -0.5*|h_j|^2
        hsq = mp.tile([128, NPER], f32, tag="hsq")
        nc.scalar.square(hsq[:], hT[:])
        sqp = pa.tile([8, NPER], f32, tag="ps_aux")
        nc.tensor.matmul(sqp[:], c["ones16bd"][:], hsq[:])
        negsq = sp.tile([8, NPER], f32, tag="negsq")
        nc.scalar.mul(negsq[:], sqp[:], -0.5)

        # A = h@(Wc_top-Wc_bot), B = h@Wc_bot   (transposed stacks [128,512])
        ap_ = pa.tile([128, NPER], f32, tag="ps_aux")
        nc.tensor.matmul(ap_[:], c["wabd"][:], hT[:])
        asb = mp.tile([128, NPER], f32, tag="Asb")
        nc.scalar.copy(asb[:], ap_[:])
        bp = pa.tile([128, NPER], f32, tag="ps_aux")
        nc.tensor.matmul(bp[:], c["wbbd"][:], hT[:])
        bt = mp.tile([128, NPER], f32, tag="BT")
        nc.scalar.copy(bt[:], bp[:])

        # PE operands must be 32-partition aligned: re-lay h per event at
        # partition base 0 (event e in columns 512e..512e+512), augmented:
        # hteS row16 = ones (stationary), hteM row16 = -|h_j|^2/2 (moving),
        # so one K=17 fp32 matmul computes rank = G - sq_j/2 exactly.
        hteS = mp.tile([17, EVB * NPER], f32, tag="hTeS")
        hteM = mp.tile([17, EVB * NPER], f32, tag="hTeM")
        for e in range(EVB):
            nc.sync.dma_start(out=hteS[0:16, NPER * e:NPER * (e + 1)],
                              in_=hT[16 * e:16 * e + 16, :])
            nc.sync.dma_start(out=hteM[0:16, NPER * e:NPER * (e + 1)],
                              in_=hT[16 * e:16 * e + 16, :])
        nc.sync.dma_start(out=hteS[16:17, :], in_=c["ones4096"][:])
        nc.sync.dma_start(out=hteM[16:17, :], in_=negsq[:])

        # per-event kNN: rank = G - |h_j|^2/2 in PSUM, top-8 via Max8/FindIndex8
        idxw = mp.tile([128, 256], u16, tag="idxw")
        for e in range(EVB):
            he = hteM[:, NPER * e:NPER * (e + 1)]
            # idx layout per event: [128 rows, 8 k, 4 t] elements at 4k+t
            idx = sp.tile([128, 32], u16, tag="idx")
            idx3 = idx[:].rearrange("p (k t) -> p k t", t=4)
            for t in range(4):
                rk = pg.tile([128, NPER], f32, tag="rank")
                nc.tensor.matmul(
                    rk[:],
                    hteS[:, NPER * e + 128 * t:NPER * e + 128 * (t + 1)],
                    he, start=True, stop=True)
                v8 = sp.tile([128, 8], f32, tag="v8")
                nc.vector.max(out=v8[:], in_=rk[:])
                nc.vector.max_index(out=idx3[:, :, t], in_max=v8[:], in_values=rk[:])
            # shuffle [128 rows, 32] u16 -> wrapped [16, 256] stripe via DRAM
            bounce = dp.tile([4096], u16, tag="bounce")
            nc.sync.dma_start(
                out=bounce[:].rearrange("(r m) -> r m", m=32), in_=idx[:])
            src = bounce[:].rearrange("(q p m) -> p q m", p=16, q=8)
            dst = idxw[16 * e:16 * e + 16, :].rearrange("p (q m) -> p q m", m=32)
            nc.sync.dma_start(out=dst, in_=src)

        # gather B rows of the 8 neighbors for every row, all 8 events at once
        gath = mp.tile([128, 4096], f32, tag="gath")
        nc.gpsimd.ap_gather(
            out_ap=gath[:], in_ap=bt[:], idxs_ap=idxw[:].bitcast(i16),
            channels=128, num_elems=NPER, d=1, num_idxs=4096)
        # segmented max over k (stride 64), un-permuting to natural i order
        # gath flat m = 512*q2 + 64*k + 16*t + p ; i = 128*t + 16*q2 + p
        gin = gath[:].rearrange("c (q k t p) -> c q t p k", q=8, k=8, t=4, p=16)
        gmax = mp.tile([128, NPER], f32, tag="gmax")
        gout = gmax[:].rearrange("c (t q p) -> c q t p", t=4, q=8, p=16)
        nc.vector.tensor_reduce(out=gout, in_=gin, axis=mybir.AxisListType.X,
                                op=OP.max)
        # f' = elu(A + gmax + bc') + 1
        z = mp.tile([128, NPER], f32, tag="zf")
        nc.vector.scalar_tensor_tensor(
            out=z[:], in0=asb[:], scalar=0.0, in1=gmax[:], op0=OP.add, op1=OP.add)
        return shifted_elu(z[:], c["bcpcol"][:], "f")

    # ---- per-block pipeline ---------------------------------------------
    for blk in range(NBLK):
        hT = encoder(blk)
        f1 = conv(hT, blk, 0)
        f2 = conv(f1, blk, 1)
        # per-event sum over particles -> pooled'(sum);  mean & -1 shift are
        # folded into the head weights/biases host-side.
        pool_t = sp.tile([128, 1], f32, tag="pool")
        nc.vector.tensor_reduce(out=pool_t[:], in_=f2[:],
                                axis=mybir.AxisListType.X, op=OP.add)
        pb = dp.tile([128], f32, tag="pbounce")
        nc.sync.dma_start(out=pb[:].rearrange("(r m) -> r m", m=1), in_=pool_t[:])
        # scatter [16e+h] -> pooledT[h, 8*blk+e]
        src_ap = bass.AP(tensor=pb[:].tensor, offset=pb[:].offset,
                         ap=[[1, 16], [16, 8], [1, 1]])
        d0 = pooledT[:, 8 * blk:8 * blk + 8]
        dst_ap = bass.AP(tensor=d0.tensor, offset=d0.offset,
                         ap=[d0.ap[0], [1, 8], [1, 1]])
        nc.sync.dma_start(out=dst_ap, in_=src_ap)

    # ---- head: 16->8->4->1 on [16,32] ------------------------------------
    o1p = pa.tile([8, 32], f32, tag="ps_aux")
    nc.tensor.matmul(o1p[:], c["wo1s"][:], pooledT[:])
    o1 = shifted_elu(o1p[:], c["bo1p"][:], "o1")
    o2p = pa.tile([4, 32], f32, tag="ps_aux")
    nc.tensor.matmul(o2p[:], c["wo2"][:], o1[:])
    o2 = shifted_elu(o2p[:], c["bo2p"][:], "o2")
    o3p = pa.tile([1, 32], f32, tag="ps_aux")
    nc.tensor.matmul(o3p[:], c["wo3"][:], o2[:])
    outsb = sp.tile([1, 32], f32, tag="outsb")
    nc.scalar.activation(outsb[:], o3p[:], AF.Identity, bias=c["bo3p"][:],
                         scale=1.0)
    nc.sync.dma_start(out=out_dram.rearrange("(a b) -> a b", a=1), in_=outsb[:])


# ---------------------------------------------------------------------------
# host side
# ---------------------------------------------------------------------------

def _bd(m, reps=8):
    """block-diagonal replication."""
    r, c_ = m.shape
    out = np.zeros((r * reps, c_ * reps), np.float32)
    for i in range(reps):
        out[i * r:(i + 1) * r, i * c_:(i + 1) * c_] = m
    return out


def make_inputs(x_pf, W1, b1, W2, b2, Wc, bc, Wo1, bo1, Wo2, bo2, Wo3, bo3):
    """Build the per-core in_maps for run_bass_kernel_spmd."""
    f = np.float32
    W1, b1, W2, b2 = (np.asarray(a, f) for a in (W1, b1, W2, b2))
    Wc, bc = np.asarray(Wc, f), np.asarray(bc, f)
    Wo1, bo1, Wo2, bo2 = (np.asarray(a, f) for a in (Wo1, bo1, Wo2, bo2))
    Wo3, bo3 = np.asarray(Wo3, f), np.asarray(bo3, f)
    x = np.asarray(x_pf, f)

    wct, wcb = Wc[:H], Wc[H:]
    consts = {
        "w1bd": _bd(W1), "w2bd": _bd(W2), "wabd": _bd(wct - wcb),
        "wbbd": _bd(wcb), "ones16bd": _bd(np.ones((16, 1), f)),
        "onesrow": np.ones((1, 128), f),
        "ones4096": np.ones((1, EVB * NPER), f),
        "b1col": np.tile(b1, NCORES)[:, None].astype(f),
        "b2pcol": np.tile(b2 - W2.sum(0), NCORES)[:, None].astype(f),
        "bcpcol": np.tile(bc - wct.sum(0), NCORES)[:, None].astype(f),
        "wo1s": (Wo1 / NPER).astype(f),
        "wo2": Wo2, "wo3": Wo3,
        "bo1p": (bo1 - Wo1.sum(0))[:, None].astype(f),
        "bo2p": (bo2 - Wo2.sum(0))[:, None].astype(f),
        "bo3p": (bo3 - Wo3.sum(0))[:, None].astype(f),
    }
    xev = x.reshape(B, NPER, 4)
    in_maps = []
    for core in range(NCORES):
        xc = xev[core * EPC:(core + 1) * EPC]                 # [32,512,4]
        xtc = xc.reshape(NBLK, EVB, NPER, 4).transpose(0, 1, 3, 2)
        xtc = np.ascontiguousarray(xtc.reshape(NBLK, 32, NPER), f)
        m = {"xt": xtc}
        m.update(consts)
        in_maps.append(m)
    return in_maps


@functools.cache
def _built_nc():
    return build_nc()


LAST_RESULT = None


def kernel(x_pf, batch_pf, W1, b1, W2, b2, Wc, bc,
           Wo1, bo1, Wo2, bo2, Wo3, bo3):
    global LAST_RESULT
    import os
    from concourse.bass_utils import run_bass_kernel_spmd
    nc = _built_nc()
    in_maps = make_inputs(x_pf, W1, b1, W2, b2, Wc, bc,
                          Wo1, bo1, Wo2, bo2, Wo3, bo3)
    kw = {}
    if os.environ.get("BASS_KERNEL_TRACE"):
        kw = dict(trace=True,
                  tmpdir=os.environ.get("BASS_KERNEL_TRACE_DIR") or None)
    res = run_bass_kernel_spmd(nc, in_maps, core_ids=list(range(NCORES)), **kw)
    LAST_RESULT = res
    out = np.concatenate([res.results[i]["out"] for i in range(NCORES)])
    return out.reshape(B, 1).astype(np.float32), np.arange(B, dtype=np.int32)
